# revision 5
# baseline (speedup 1.0000x reference)
"""GNN aggregator kernel for Trainium2 (8 NeuronCores, SPMD, no collectives).

Math (per reference):
    H[v]         = sum_{e: dst[e]==v} entity_embed[src[e]]
    neighbors    = leaky_relu(H @ W1.T + b1)          (slope 0.01)
    interactions = leaky_relu((entity_embed * H) @ W2.T + b2)
    out          = neighbors + interactions

Sharding: edges are partitioned by dst node range. Core c owns nodes
[c*6250, (c+1)*6250) and receives exactly the edges whose dst falls in its
range, so each core computes its output slice with no collectives.

Per core, edges are bucketed by dst block (128 nodes per block, 49 blocks)
and, within a bucket, split by src half (so gather indices fit in int16) and
padded to a uniform count. The segment-sum becomes dense matmuls:

    HT_block[f, n] = sum_t  msgs_t[e, f].T @ onehot_t[e, n]

where msgs_t is a 128-edge tile of gathered src embeddings (dma_gather) and
onehot_t[e, n] = (r_e == n) is built on the vector engine from an iota tile
(r_e = dst position within the block; padding edges get r = -1 => zero row).
The two 128x128 linears then run per block on HT without any transposes, and
the output is written feature-major; the host transposes slices back.
"""

import sys

sys.path.insert(0, "/opt/trn_rl_repo")

import numpy as np

import concourse.bass as bass
import concourse.tile as tile
from concourse import bacc, mybir

P = 128
N_NODES = 50000
D = 128
N_CORES = 8
PER_CORE = N_NODES // N_CORES  # 6250
HALF = (N_NODES + 1) // 2      # 25000: src table split so indices fit int16
LEAKY = 0.01

F32 = mybir.dt.float32
I16 = mybir.dt.int16


def host_prep(entity_embed, src, dst, n_cores, per_core, half):
    """Shard + layout inputs for the SPMD kernel.

    Returns (cfg, per_core_inmaps_common_arrays).
    """
    entity_embed = np.asarray(entity_embed, dtype=np.float32)
    src = np.asarray(src).astype(np.int64)
    dst = np.asarray(dst).astype(np.int64)
    n_nodes = entity_embed.shape[0]
    n_blocks = (per_core + P - 1) // P

    core = dst // per_core
    local = dst - core * per_core
    q = local >> 7
    r = (local & 127).astype(np.float32)
    halfv = (src >= half).astype(np.int64)
    idx16 = (src - halfv * half).astype(np.int16)

    ngroup = n_cores * n_blocks * 2
    okey = (core * n_blocks + q) * 2 + halfv
    counts = np.bincount(okey, minlength=ngroup)
    h_pad = max(int((counts.max() + P - 1) // P) * P, P)
    n_th = h_pad // P  # 128-edge tiles per (bucket, half)

    order = np.argsort(okey, kind="stable")
    okey_s = okey[order]
    starts = np.zeros(ngroup, np.int64)
    np.cumsum(counts[:-1], out=starts[1:])
    pos = np.arange(len(src), dtype=np.int64) - starts[okey_s]
    flat = okey_s * h_pad + pos

    idx_flat = np.zeros(ngroup * h_pad, np.int16)
    idx_flat[flat] = idx16[order]
    rv_flat = np.full(ngroup * h_pad, -1.0, np.float32)
    rv_flat[flat] = r[order]

    idx_arr = idx_flat.reshape(n_cores, n_blocks, 2, h_pad)
    rv_arr = rv_flat.reshape(n_cores, n_blocks, 2, h_pad)

    # Gather-index layout: element j of a call lives at [j%16, j//16],
    # replicated across the 8 16-partition groups.
    wrapped = idx_arr.reshape(n_cores, n_blocks, 2, h_pad // 16, 16)
    wrapped = np.swapaxes(wrapped, -1, -2)  # [..., 16, h_pad//16]
    wrapped = np.tile(wrapped, (1, 1, 1, 8, 1))  # -> [..., 128, h_pad//16]
    idx_dram = np.ascontiguousarray(
        np.concatenate([wrapped[:, :, 0], wrapped[:, :, 1]], axis=-1)
    )  # [n_cores, n_blocks, 128, 2*(h_pad//16)]

    # r-value layout: edge slot j of tile t -> partition j%128, column t.
    rv_t = np.swapaxes(rv_arr.reshape(n_cores, n_blocks, 2, n_th, P), -1, -2)
    # [c, q, h, p, t] -> [c, p, q, h, t] -> [c, 128, n_blocks*2*n_th]
    rv_dram = np.ascontiguousarray(
        rv_t.transpose(0, 3, 1, 2, 4).reshape(n_cores, P, n_blocks * 2 * n_th)
    )

    # Per-core transposed embedding slice, padded to n_blocks*128 columns.
    et_own = np.zeros((n_cores, P, n_blocks * P), np.float32)
    for c in range(n_cores):
        sl = entity_embed[c * per_core : (c + 1) * per_core]
        et_own[c, :, : sl.shape[0]] = sl.T

    etab_lo = np.ascontiguousarray(entity_embed[:half])
    etab_hi = np.ascontiguousarray(entity_embed[half:])
    if etab_hi.shape[0] < half:  # pad to uniform half size
        pad = np.zeros((half - etab_hi.shape[0], entity_embed.shape[1]), np.float32)
        etab_hi = np.concatenate([etab_hi, pad], axis=0)

    cfg = dict(
        n_blocks=n_blocks,
        n_th=n_th,
        h_pad=h_pad,
        half=half,
        per_core=per_core,
        d=entity_embed.shape[1],
    )
    arrays = dict(
        idx=idx_dram, rv=rv_dram, et_own=et_own,
        etab_lo=etab_lo, etab_hi=etab_hi,
    )
    return cfg, arrays


def build_program(cfg, n_cores):
    n_blocks = cfg["n_blocks"]
    n_th = cfg["n_th"]
    h_pad = cfg["h_pad"]
    half = cfg["half"]
    d = cfg["d"]
    idx_cols = 2 * (h_pad // 16)
    ntiles = 2 * n_th  # edge tiles per bucket

    nc = bacc.Bacc("TRN2", target_bir_lowering=False, debug=False,
                   num_devices=n_cores)

    etab_lo = nc.dram_tensor("etab_lo", [half, d], F32, kind="ExternalInput")
    etab_hi = nc.dram_tensor("etab_hi", [half, d], F32, kind="ExternalInput")
    et_own = nc.dram_tensor("et_own", [P, n_blocks * P], F32, kind="ExternalInput")
    w1t = nc.dram_tensor("w1t", [d, d], F32, kind="ExternalInput")
    w2t = nc.dram_tensor("w2t", [d, d], F32, kind="ExternalInput")
    b1 = nc.dram_tensor("b1", [d, 1], F32, kind="ExternalInput")
    nb1 = nc.dram_tensor("nb1", [d, 1], F32, kind="ExternalInput")
    b2 = nc.dram_tensor("b2", [d, 1], F32, kind="ExternalInput")
    nb2 = nc.dram_tensor("nb2", [d, 1], F32, kind="ExternalInput")
    iota = nc.dram_tensor("iota", [P, P], F32, kind="ExternalInput")
    idx = nc.dram_tensor("idx", [n_blocks, P, idx_cols], I16, kind="ExternalInput")
    rv = nc.dram_tensor("rv", [P, n_blocks * ntiles * P // P], F32,
                        kind="ExternalInput")
    outT = nc.dram_tensor("outT", [P, n_blocks * P], F32, kind="ExternalOutput")

    with tile.TileContext(nc) as tc:
        with (
            tc.tile_pool(name="static", bufs=1) as static,
            tc.tile_pool(name="idxp", bufs=3) as idxp,
            tc.tile_pool(name="msgp", bufs=3) as msgp,
            tc.tile_pool(name="ohp", bufs=4) as ohp,
            tc.tile_pool(name="htp", bufs=3) as htp,
            tc.tile_pool(name="postp", bufs=6) as postp,
            tc.tile_pool(name="psacc", bufs=2, space="PSUM") as psacc,
            tc.tile_pool(name="psmlp", bufs=2, space="PSUM") as psmlp,
        ):
            et_own_t = static.tile([P, n_blocks * P], F32)
            nc.sync.dma_start(et_own_t[:], et_own.ap())
            rv_t = static.tile([P, n_blocks * ntiles], F32)
            nc.sync.dma_start(rv_t[:], rv.ap())
            iota_t = static.tile([P, P], F32)
            nc.sync.dma_start(iota_t[:], iota.ap())
            w1t_t = static.tile([d, d], F32)
            nc.sync.dma_start(w1t_t[:], w1t.ap())
            w2t_t = static.tile([d, d], F32)
            nc.sync.dma_start(w2t_t[:], w2t.ap())
            b1_t = static.tile([d, 1], F32)
            nc.sync.dma_start(b1_t[:], b1.ap())
            nb1_t = static.tile([d, 1], F32)
            nc.sync.dma_start(nb1_t[:], nb1.ap())
            b2_t = static.tile([d, 1], F32)
            nc.sync.dma_start(b2_t[:], b2.ap())
            nb2_t = static.tile([d, 1], F32)
            nc.sync.dma_start(nb2_t[:], nb2.ap())

            relu = mybir.ActivationFunctionType.Relu

            for q in range(n_blocks):
                idx_t = idxp.tile([P, idx_cols], I16)
                nc.sync.dma_start(idx_t[:], idx.ap()[q, :, :])

                msgs = msgp.tile([P, ntiles * P], F32)
                lo_out = msgs[:, : n_th * P].rearrange("p (t e) -> p t e", e=P)
                hi_out = msgs[:, n_th * P :].rearrange("p (t e) -> p t e", e=P)
                nc.gpsimd.dma_gather(
                    lo_out, etab_lo.ap(), idx_t[:, : idx_cols // 2],
                    num_idxs=h_pad, num_idxs_reg=h_pad, elem_size=d,
                    single_packet=False,
                )
                nc.gpsimd.dma_gather(
                    hi_out, etab_hi.ap(), idx_t[:, idx_cols // 2 :],
                    num_idxs=h_pad, num_idxs_reg=h_pad, elem_size=d,
                    single_packet=False,
                )

                ht_ps = psacc.tile([P, P], F32)
                for t in range(ntiles):
                    oh = ohp.tile([P, P], F32)
                    col = q * ntiles + t
                    nc.vector.tensor_scalar(
                        oh[:], iota_t[:], rv_t[:, col : col + 1], None,
                        mybir.AluOpType.is_equal,
                    )
                    nc.tensor.matmul(
                        ht_ps[:], lhsT=msgs[:, t * P : (t + 1) * P], rhs=oh[:],
                        start=(t == 0), stop=(t == ntiles - 1),
                    )

                ht = htp.tile([P, P], F32)
                nc.vector.tensor_copy(ht[:], ht_ps[:])
                tmp = htp.tile([P, P], F32)
                nc.vector.tensor_mul(
                    tmp[:], ht[:], et_own_t[:, q * P : (q + 1) * P]
                )

                nt_ps = psmlp.tile([P, P], F32)
                nc.tensor.matmul(nt_ps[:], lhsT=w1t_t[:], rhs=ht[:],
                                 start=True, stop=True)
                it_ps = psmlp.tile([P, P], F32)
                nc.tensor.matmul(it_ps[:], lhsT=w2t_t[:], rhs=tmp[:],
                                 start=True, stop=True)

                # leaky(x+b) = relu(x+b) - slope*relu(-(x+b))
                a1 = postp.tile([P, P], F32)
                nc.scalar.activation(a1[:], nt_ps[:], relu, bias=b1_t[:], scale=1.0)
                n1 = postp.tile([P, P], F32)
                nc.scalar.activation(n1[:], nt_ps[:], relu, bias=nb1_t[:], scale=-1.0)
                a2 = postp.tile([P, P], F32)
                nc.scalar.activation(a2[:], it_ps[:], relu, bias=b2_t[:], scale=1.0)
                n2 = postp.tile([P, P], F32)
                nc.scalar.activation(n2[:], it_ps[:], relu, bias=nb2_t[:], scale=-1.0)

                s1 = postp.tile([P, P], F32)
                nc.vector.tensor_add(s1[:], a1[:], a2[:])
                s2 = postp.tile([P, P], F32)
                nc.vector.tensor_add(s2[:], n1[:], n2[:])
                s2m = postp.tile([P, P], F32)
                nc.vector.tensor_scalar_mul(s2m[:], s2[:], -LEAKY)
                ob = postp.tile([P, P], F32)
                nc.vector.tensor_add(ob[:], s1[:], s2m[:])

                nc.sync.dma_start(outT.ap()[:, q * P : (q + 1) * P], ob[:])

    nc.compile()
    return nc


def make_inmaps(cfg, arrays, W1, b1, W2, b2, n_cores):
    d = cfg["d"]
    W1 = np.asarray(W1, dtype=np.float32)
    W2 = np.asarray(W2, dtype=np.float32)
    b1 = np.asarray(b1, dtype=np.float32).reshape(d, 1)
    b2 = np.asarray(b2, dtype=np.float32).reshape(d, 1)
    iota = np.tile(np.arange(P, dtype=np.float32)[None, :], (P, 1))
    common = dict(
        etab_lo=arrays["etab_lo"], etab_hi=arrays["etab_hi"],
        w1t=np.ascontiguousarray(W1.T), w2t=np.ascontiguousarray(W2.T),
        b1=b1, nb1=-b1, b2=b2, nb2=-b2, iota=iota,
    )
    in_maps = []
    for c in range(n_cores):
        m = dict(common)
        m["et_own"] = arrays["et_own"][c]
        m["idx"] = arrays["idx"][c]
        m["rv"] = arrays["rv"][c]
        in_maps.append(m)
    return in_maps


_cache = {}


def run(entity_embed, W1, b1, W2, b2, src, dst, trace=False, trace_kwargs=None):
    from concourse.bass_utils import run_bass_kernel_spmd

    cfg, arrays = host_prep(entity_embed, src, dst, N_CORES, PER_CORE, HALF)
    key = (cfg["n_blocks"], cfg["n_th"], cfg["h_pad"], cfg["half"], cfg["d"])
    if key not in _cache:
        _cache[key] = build_program(cfg, N_CORES)
    nc = _cache[key]

    in_maps = make_inmaps(cfg, arrays, W1, b1, W2, b2, N_CORES)
    res = run_bass_kernel_spmd(
        nc, in_maps, core_ids=list(range(N_CORES)),
        trace=trace, **(dict(trace_kwargs=trace_kwargs) if trace_kwargs else {}),
    )

    out = np.empty((N_NODES, D), np.float32)
    for c in range(N_CORES):
        out[c * PER_CORE : (c + 1) * PER_CORE] = \
            res.results[c]["outT"][:, :PER_CORE].T
    return out, res


def kernel(entity_embed, W1, b1, W2, b2, src, dst):
    out, _ = run(entity_embed, W1, b1, W2, b2, src, dst)
    return out
